# revision 15
# baseline (speedup 1.0000x reference)
"""Trainium2 Bass kernel for nn_AttnGCN (2-layer GATv2 + BN + dropout + FC).

Sharding: nodes are partitioned across 8 NeuronCores (graph parallel).  Each
core owns a contiguous range of 6250 destination nodes (padded to 6272 =
49*128).  Edges are bucketed by destination tile on the host (index-only
preprocessing), each tile's edge list padded to whole 128-edge blocks.  Layer-1
runs per-core on the edge shard; BN statistics are combined with a tiny
AllReduce; the activated layer-1 features are AllGathered so every core can
gather arbitrary source rows for layer-2; layer-2 + FC produce the owned output
shard, which the host concatenates.

All numeric work (matmuls, softmax, scatter/gather, BN, masking) happens on
device.  Host does only index bucketing, parameter layout, and output
reassembly.  Dropout masks are the fixed jax PRNG streams of the reference
(input-independent constants), computed once on host CPU.
"""

import os
import sys
import types
import numpy as np

import concourse.bacc as bacc
import concourse.bass as bass
import concourse.mybir as mybir
import concourse.tile as tile
from concourse.bass_utils import run_bass_kernel_spmd
from concourse.masks import make_identity

P = 128
NCORES = 8
N = 50000
E = 400000
NCF = 26          # input/output feature dim
H = 2
CH = 128
HC = 256
NOWN = N // NCORES            # 6250 owned nodes per core
NT = (NOWN + P - 1) // P      # 49 node tiles per core
NPC = NT * P                  # 6272 padded nodes per core
NPAD = NCORES * NPC           # 50176
NREAL_LAST = NOWN - (NT - 1) * P   # 106 real nodes in last tile
DP_SCALE = 1.25               # 1/(1-0.2)

FP32 = mybir.dt.float32
F32R = mybir.dt.float32r
I32 = mybir.dt.int32
AF = mybir.ActivationFunctionType
OP = mybir.AluOpType
RG = [list(range(NCORES))]

_PROGRAM_CACHE = {}
_MASK_CACHE = {}

_MASK_SCRIPT = r"""
import os, sys
for _p in reversed(os.environ.get("NIX_PYTHONPATH", "").split(os.pathsep)):
    if _p and _p not in sys.path:
        sys.path.insert(0, _p)
import numpy as np
import jax
m1 = np.asarray(jax.random.bernoulli(jax.random.key(1), 0.8, (%d, %d)),
                dtype=np.float32)
m2 = np.asarray(jax.random.bernoulli(jax.random.key(2), 0.8, (%d, %d)),
                dtype=np.float32)
np.savez(sys.argv[1], m1=m1, m2=m2)
"""


def _dropout_masks():
    """Reference dropout masks: fixed jax PRNG streams, computed with plain
    CPU jax (subprocess) so the bit stream matches a stock jax environment."""
    if "m" not in _MASK_CACHE:
        import subprocess
        import tempfile
        env = dict(os.environ)
        env["JAX_PLATFORMS"] = "cpu"
        env.pop("XLA_FLAGS", None)
        env.pop("TRN_TERMINAL_POOL_IPS", None)
        with tempfile.TemporaryDirectory() as td:
            fn = os.path.join(td, "masks.npz")
            script = _MASK_SCRIPT % (N, HC, N, HC)
            r = subprocess.run([sys.executable, "-c", script, fn], env=env,
                               capture_output=True, text=True)
            if r.returncode != 0:
                raise RuntimeError("mask subprocess failed: " + r.stderr[-2000:])
            d = np.load(fn)
            _MASK_CACHE["m"] = (d["m1"] * DP_SCALE, d["m2"] * DP_SCALE)
    return _MASK_CACHE["m"]


# ----------------------------------------------------------------------------
# host-side index preprocessing (sharding)
# ----------------------------------------------------------------------------

def _bucket_edges(dst, NBT, off, order, cnt, bounds, fill_fn):
    """Place sorted-by-(core,tile) edges into the uniform padded block layout."""
    for r in range(NCORES):
        for t in range(NT):
            k = r * NT + t
            c = cnt[r, t]
            if c == 0:
                continue
            sl = order[bounds[k]:bounds[k] + c]
            base = off[t] * P
            fill_fn(r, base, c, sl)


def _host_prep(x_input, edge_weight, params, edge_index):
    src = np.asarray(edge_index[0], dtype=np.int64).astype(np.int32)
    dst = np.asarray(edge_index[1], dtype=np.int64).astype(np.int32)
    w = np.asarray(edge_weight, dtype=np.float32).reshape(-1)
    x = np.asarray(x_input, dtype=np.float32)

    r_arr = dst // NOWN
    nloc = dst - r_arr * NOWN
    t_arr = nloc >> 7
    dstloc = (nloc & 127).astype(np.int32)

    key = r_arr * NT + t_arr
    order = np.argsort(key, kind="stable")
    cnt = np.bincount(key, minlength=NCORES * NT).reshape(NCORES, NT)
    bounds = np.concatenate([[0], np.cumsum(cnt.reshape(-1))]).astype(np.int64)

    # ----- layer 1 blocks -----
    NBT1 = np.maximum(1, -(-cnt.max(0) // P)).astype(np.int64)
    off1 = np.concatenate([[0], np.cumsum(NBT1[:-1])]).astype(np.int64)
    LB1 = int(NBT1.sum())
    LE1 = LB1 * P
    edges1 = np.zeros((NCORES, LE1, 3), np.int32)
    edges1[:, :, 1] = 255

    def fill1(r, base, c, sl):
        edges1[r, base:base + c, 0] = src[sl]
        edges1[r, base:base + c, 1] = dstloc[sl]
        edges1[r, base:base + c, 2] = w[sl].view(np.int32)

    _bucket_edges(dst, NBT1, off1, order, cnt, bounds, fill1)

    # ----- layer 2 blocks (real edges + self loops) -----
    nreal = np.full(NT, P, np.int64)
    nreal[NT - 1] = NREAL_LAST
    cnt2 = cnt + nreal[None, :]
    NBT2 = np.maximum(1, -(-cnt2.max(0) // P)).astype(np.int64)
    off2 = np.concatenate([[0], np.cumsum(NBT2[:-1])]).astype(np.int64)
    LB2 = int(NBT2.sum())
    LE2 = LB2 * P
    LE2P = LE2 + P
    g2src = (src // NOWN) * NPC + src % NOWN   # padded-global source ids

    edges2 = np.zeros((NCORES, LE2, 2), np.int32)
    edges2[:, :, 1] = 255
    w2x = np.zeros((NCORES, LE2P), np.float32)
    slots2 = np.zeros((NCORES, NT * P), np.int32)

    for r in range(NCORES):
        for t in range(NT):
            k = r * NT + t
            c = int(cnt[r, t])
            sl = order[bounds[k]:bounds[k] + c]
            base = int(off2[t]) * P
            edges2[r, base:base + c, 0] = g2src[sl]
            edges2[r, base:base + c, 1] = dstloc[sl]
            w2x[r, base:base + c] = w[sl]
            nr = int(nreal[t])
            # self loops for the tile's real nodes
            pos = base + c
            edges2[r, pos:pos + nr, 0] = r * NPC + t * P + np.arange(nr)
            edges2[r, pos:pos + nr, 1] = np.arange(nr)
            slots2[r, t * P:t * P + nr] = pos + np.arange(nr)
            # pad nodes' loop-attr scatter goes to the dummy tail area
            slots2[r, t * P + nr:(t + 1) * P] = LE2 + np.arange(nr, P)

    # ----- degrees / masks / params -----
    deg = np.bincount(dst, minlength=N).astype(np.float32)
    invdeg_full = 1.0 / np.maximum(deg, 1.0)
    invdeg = np.ones((NCORES, NPC), np.float32)
    for r in range(NCORES):
        invdeg[r, :NOWN] = invdeg_full[r * NOWN:(r + 1) * NOWN]

    m1, m2 = _dropout_masks()

    def shard_rows(a):
        out = np.zeros((NCORES, NPC) + a.shape[1:], np.float32)
        for r in range(NCORES):
            out[r, :NOWN] = a[r * NOWN:(r + 1) * NOWN]
        return out

    mask1 = shard_rows(m1)
    mask2 = shard_rows(m2)
    x_own = shard_rows(x)

    p = {k: np.asarray(v, dtype=np.float32) for k, v in params.items()}
    wfc_pk = np.zeros((P, 52), np.float32)
    wfc_pk[:, :26] = p["Wfc"][:128]
    wfc_pk[:, 26:] = p["Wfc"][128:]

    common = {
        "x": x,
        "Wl1": p["Wl1"], "Wr1": p["Wr1"],
        "we1r": np.tile(p["We1"].reshape(1, HC), (P, 1)),
        "att1r": np.tile(p["att1"].reshape(1, HC), (P, 1)),
        "Wl2": p["Wl2"], "Wr2": p["Wr2"],
        "we2r": np.tile(p["We2"].reshape(1, HC), (P, 1)),
        "att2r": np.tile(p["att2"].reshape(1, HC), (P, 1)),
        "wfc": wfc_pk,
        "bfcr": np.tile(p["bfc"].reshape(1, NCF), (P, 1)),
        "g1row": p["g1"].reshape(1, HC), "be1row": p["be1"].reshape(1, HC),
        "g2row": p["g2"].reshape(1, HC), "be2row": p["be2"].reshape(1, HC),
    }
    in_maps = []
    for r in range(NCORES):
        m = dict(common)
        m["edges1"] = edges1[r]
        m["edges2"] = edges2[r]
        m["w2x"] = w2x[r][:, None]
        m["slots2"] = slots2[r][:, None]
        m["invdeg"] = invdeg[r][:, None]
        m["mask1"] = mask1[r]
        m["mask2"] = mask2[r]
        m["x_own"] = x_own[r]
        in_maps.append(m)

    meta = dict(NBT1=tuple(int(v) for v in NBT1), off1=tuple(int(v) for v in off1),
                NBT2=tuple(int(v) for v in NBT2), off2=tuple(int(v) for v in off2),
                LE1=LE1, LE2=LE2, LE2P=LE2P)
    return in_maps, meta


# ----------------------------------------------------------------------------
# device program
# ----------------------------------------------------------------------------

def _build_program(meta):
    NBT1, off1 = meta["NBT1"], meta["off1"]
    NBT2, off2 = meta["NBT2"], meta["off2"]
    LE1, LE2, LE2P = meta["LE1"], meta["LE2"], meta["LE2P"]

    nc = bacc.Bacc("TRN2", target_bir_lowering=False)

    # ---- I/O ----
    xT = nc.dram_tensor("x", (N, NCF), FP32, kind="ExternalInput")
    e1T = nc.dram_tensor("edges1", (LE1, 3), I32, kind="ExternalInput")
    e2T = nc.dram_tensor("edges2", (LE2, 2), I32, kind="ExternalInput")
    w2xT = nc.dram_tensor("w2x", (LE2P, 1), FP32, kind="ExternalInput")
    slotsT = nc.dram_tensor("slots2", (NT * P, 1), I32, kind="ExternalInput")
    invdT = nc.dram_tensor("invdeg", (NPC, 1), FP32, kind="ExternalInput")
    mask1T = nc.dram_tensor("mask1", (NPC, HC), FP32, kind="ExternalInput")
    mask2T = nc.dram_tensor("mask2", (NPC, HC), FP32, kind="ExternalInput")
    xownT = nc.dram_tensor("x_own", (NPC, NCF), FP32, kind="ExternalInput")
    Wl1T = nc.dram_tensor("Wl1", (NCF, HC), FP32, kind="ExternalInput")
    Wr1T = nc.dram_tensor("Wr1", (NCF, HC), FP32, kind="ExternalInput")
    we1rT = nc.dram_tensor("we1r", (P, HC), FP32, kind="ExternalInput")
    att1rT = nc.dram_tensor("att1r", (P, HC), FP32, kind="ExternalInput")
    Wl2T = nc.dram_tensor("Wl2", (HC, HC), FP32, kind="ExternalInput")
    Wr2T = nc.dram_tensor("Wr2", (HC, HC), FP32, kind="ExternalInput")
    we2rT = nc.dram_tensor("we2r", (P, HC), FP32, kind="ExternalInput")
    att2rT = nc.dram_tensor("att2r", (P, HC), FP32, kind="ExternalInput")
    wfcT = nc.dram_tensor("wfc", (P, 52), FP32, kind="ExternalInput")
    bfcrT = nc.dram_tensor("bfcr", (P, NCF), FP32, kind="ExternalInput")
    g1rT = nc.dram_tensor("g1row", (1, HC), FP32, kind="ExternalInput")
    be1rT = nc.dram_tensor("be1row", (1, HC), FP32, kind="ExternalInput")
    g2rT = nc.dram_tensor("g2row", (1, HC), FP32, kind="ExternalInput")
    be2rT = nc.dram_tensor("be2row", (1, HC), FP32, kind="ExternalInput")
    outT = nc.dram_tensor("out", (NPC, NCF), FP32, kind="ExternalOutput")

    # ---- internal DRAM ----
    dbg = bool(os.environ.get("KERNEL_DBG"))
    dbgkind = {"kind": "ExternalOutput"} if dbg else {}
    hpre1 = nc.dram_tensor("hpre1", (NPC, HC), FP32, **dbgkind)
    h1act = nc.dram_tensor("h1act", (NPC, HC), FP32)
    h1dbg = (nc.dram_tensor("h1dbg", (NPC, HC), FP32, kind="ExternalOutput")
             if dbg else None)
    h1full = nc.dram_tensor("h1full", (NPAD, HC), FP32, addr_space="Shared")
    hpre2 = nc.dram_tensor("hpre2", (NPC, HC), FP32, **dbgkind)
    w2i = nc.dram_tensor("w2i", (LE2P, 1), FP32, **dbgkind)
    st1i = nc.dram_tensor("st1i", (1, 2 * HC), FP32)
    st1o = nc.dram_tensor("st1o", (1, 2 * HC), FP32, addr_space="Shared")
    st2i = nc.dram_tensor("st2i", (1, 2 * HC), FP32)
    st2o = nc.dram_tensor("st2o", (1, 2 * HC), FP32, addr_space="Shared")

    with tile.TileContext(nc) as tc:
        with tc.tile_pool(name="cst", bufs=1) as cst, \
             tc.tile_pool(name="sb", bufs=3) as sb, \
             tc.tile_pool(name="mm", bufs=2, space="PSUM") as mmp, \
             tc.tile_pool(name="tr", bufs=2, space="PSUM") as trp, \
             tc.tile_pool(name="accp", bufs=2, space="PSUM") as accp:

            # ---------------- constants ----------------
            iota_i = cst.tile([P, P], I32, tag="iota_i")
            nc.gpsimd.iota(iota_i[:], pattern=[[1, P]], base=0,
                           channel_multiplier=0)
            iota_f = cst.tile([P, P], FP32, tag="iota_f")
            nc.vector.tensor_copy(iota_f[:], iota_i[:])
            ident32 = cst.tile([P, P], FP32, tag="ident32")
            make_identity(nc, ident32[:])

            def load_const(name, dram, shape, dtype=FP32, rcast=False):
                t = cst.tile(shape, dtype, tag=name)
                src_ap = dram[:, :]
                if rcast:
                    src_ap = src_ap.bitcast(F32R)
                nc.sync.dma_start(out=t[:], in_=src_ap)
                return t

            Wl1s = load_const("Wl1s", Wl1T, [NCF, HC], F32R, rcast=True)
            Wr1s = load_const("Wr1s", Wr1T, [NCF, HC], F32R, rcast=True)
            we1s = load_const("we1s", we1rT, [P, HC])
            att1s = load_const("att1s", att1rT, [P, HC])
            we2s = load_const("we2s", we2rT, [P, HC])
            att2s = load_const("att2s", att2rT, [P, HC])
            wfcs = load_const("wfcs", wfcT, [P, 52], F32R, rcast=True)
            bfcs = load_const("bfcs", bfcrT, [P, NCF])
            g1s = load_const("g1s", g1rT, [1, HC])
            be1s = load_const("be1s", be1rT, [1, HC])
            g2s = load_const("g2s", g2rT, [1, HC])
            be2s = load_const("be2s", be2rT, [1, HC])
            # Wl2/Wr2 packed as [128, 512] (two K chunks side by side)
            Wl2s = cst.tile([P, 2 * HC], F32R, tag="Wl2s")
            Wr2s = cst.tile([P, 2 * HC], F32R, tag="Wr2s")
            for kk in range(2):
                nc.sync.dma_start(out=Wl2s[:, kk * HC:(kk + 1) * HC],
                                  in_=Wl2T[kk * P:(kk + 1) * P, :].bitcast(F32R))
                nc.sync.dma_start(out=Wr2s[:, kk * HC:(kk + 1) * HC],
                                  in_=Wr2T[kk * P:(kk + 1) * P, :].bitcast(F32R))
            ones_col = cst.tile([P, 1], FP32, tag="ones_col")
            nc.vector.memset(ones_col[:], 1.0)
            ones_row = cst.tile([1, P], FP32, tag="ones_row")
            nc.vector.memset(ones_row[:], 1.0)

            # stats accumulators in SBUF
            st1sb = cst.tile([1, 2 * HC], FP32, tag="st1sb")
            nc.vector.memset(st1sb[:], 0.0)
            st2sb = cst.tile([1, 2 * HC], FP32, tag="st2sb")
            nc.vector.memset(st2sb[:], 0.0)

            # copy host edge weights for layer 2 (self-loop slots get filled
            # by the device during layer-1 finalize)
            nrows = LE2P // P
            for c0 in range(0, nrows, P):
                cn = min(P, nrows - c0)
                w2cp = sb.tile([P, P], FP32, tag="w2cp")
                nc.sync.dma_start(
                    out=w2cp[:cn, :],
                    in_=w2xT[:, 0].rearrange("(a b) -> a b", b=P)[c0:c0 + cn, :])
                nc.sync.dma_start(
                    out=w2i[:, 0].rearrange("(a b) -> a b", b=P)[c0:c0 + cn, :],
                    in_=w2cp[:cn, :])

            # ================= generic GATv2 edge layer =================
            def edge_layer(layer):
                if layer == 1:
                    NBT, off, eT = NBT1, off1, e1T
                    wes, atts = we1s, att1s
                    hpre_dram = hpre1
                    stsb = st1sb
                else:
                    NBT, off, eT = NBT2, off2, e2T
                    wes, atts = we2s, att2s
                    hpre_dram = hpre2
                    stsb = st2sb

                for t in range(NT):
                    # ---- XR tile for the 128 owned nodes ----
                    if layer == 1:
                        xo = sb.tile([P, NCF], FP32, tag="xo")
                        nc.sync.dma_start(out=xo[:],
                                          in_=xownT[t * P:(t + 1) * P, :])
                        xoT_ps = trp.tile([P, P], FP32, tag="tr")
                        nc.tensor.transpose(out=xoT_ps[:NCF, :], in_=xo[:],
                                            identity=ident32[:])
                        xoTs = sb.tile([NCF, P], F32R, tag="xoTs")
                        nc.vector.tensor_copy(xoTs[:], xoT_ps[:NCF, :])
                        xr_ps = mmp.tile([P, HC], FP32, tag="mm")
                        nc.tensor.matmul(out=xr_ps[:], lhsT=xoTs[:],
                                         rhs=Wr1s[:], start=True, stop=True)
                    else:
                        xo = sb.tile([P, HC], FP32, tag="xo2")
                        nc.sync.dma_start(out=xo[:],
                                          in_=h1act[t * P:(t + 1) * P, :])
                        xoTs = sb.tile([P, HC], F32R, tag="xoTs2")
                        for kk in range(2):
                            tp = trp.tile([P, P], FP32, tag="tr")
                            nc.tensor.transpose(
                                out=tp[:], in_=xo[:, kk * P:(kk + 1) * P],
                                identity=ident32[:])
                            nc.vector.tensor_copy(
                                xoTs[:, kk * P:(kk + 1) * P], tp[:])
                        xr_ps = mmp.tile([P, HC], FP32, tag="mm")
                        for kk in range(2):
                            nc.tensor.matmul(
                                out=xr_ps[:],
                                lhsT=xoTs[:, kk * P:(kk + 1) * P],
                                rhs=Wr2s[:, kk * HC:(kk + 1) * HC],
                                start=(kk == 0), stop=(kk == 1))
                    xr_sb = sb.tile([P, HC], F32R, tag="xr_sb")
                    nc.scalar.activation(xr_sb[:], xr_ps[:], AF.Copy)

                    acc = accp.tile([P, 260], FP32, tag="acc")
                    nb = NBT[t]
                    for b in range(nb):
                        o0 = (off[t] + b) * P
                        if layer == 1:
                            eb = sb.tile([P, 3], I32, tag="eb")
                            nc.sync.dma_start(out=eb[:], in_=e1T[o0:o0 + P, :])
                            wcol = eb[:, 2:3].bitcast(FP32)
                        else:
                            eb = sb.tile([P, 2], I32, tag="eb")
                            nc.sync.dma_start(out=eb[:], in_=e2T[o0:o0 + P, :])
                            wb = sb.tile([P, 1], FP32, tag="wb")
                            nc.sync.dma_start(out=wb[:], in_=w2i[o0:o0 + P, :])
                            wcol = wb[:, :1]

                        # ---- gather + source transform -> G [128, 256] ----
                        if layer == 1:
                            xg = sb.tile([P, NCF], FP32, tag="xg")
                            nc.gpsimd.indirect_dma_start(
                                out=xg[:], out_offset=None, in_=xT[:, :],
                                in_offset=bass.IndirectOffsetOnAxis(
                                    ap=eb[:, 0:1], axis=0))
                            xgT_ps = trp.tile([P, P], FP32, tag="tr")
                            nc.tensor.transpose(out=xgT_ps[:NCF, :], in_=xg[:],
                                                identity=ident32[:])
                            xgTs = sb.tile([NCF, P], F32R, tag="xgTs")
                            nc.vector.tensor_copy(xgTs[:], xgT_ps[:NCF, :])
                            g_ps = mmp.tile([P, HC], FP32, tag="mm")
                            nc.tensor.matmul(out=g_ps[:], lhsT=xgTs[:],
                                             rhs=Wl1s[:], start=True, stop=True)
                        else:
                            grow = sb.tile([P, HC], FP32, tag="grow")
                            nc.gpsimd.indirect_dma_start(
                                out=grow[:], out_offset=None, in_=h1full[:, :],
                                in_offset=bass.IndirectOffsetOnAxis(
                                    ap=eb[:, 0:1], axis=0))
                            gTs = sb.tile([P, HC], F32R, tag="gTs")
                            for kk in range(2):
                                tp = trp.tile([P, P], FP32, tag="tr")
                                nc.tensor.transpose(
                                    out=tp[:], in_=grow[:, kk * P:(kk + 1) * P],
                                    identity=ident32[:])
                                nc.vector.tensor_copy(
                                    gTs[:, kk * P:(kk + 1) * P], tp[:])
                            g_ps = mmp.tile([P, HC], FP32, tag="mm")
                            for kk in range(2):
                                nc.tensor.matmul(
                                    out=g_ps[:],
                                    lhsT=gTs[:, kk * P:(kk + 1) * P],
                                    rhs=Wl2s[:, kk * HC:(kk + 1) * HC],
                                    start=(kk == 0), stop=(kk == 1))
                        g_sb = sb.tile([P, HC], FP32, tag="g_sb")
                        nc.scalar.activation(g_sb[:], g_ps[:], AF.Copy)

                        # ---- one-hot by local destination ----
                        d_f = sb.tile([P, 1], FP32, tag="d_f")
                        nc.vector.tensor_copy(d_f[:], eb[:, 1:2])
                        oh = sb.tile([P, P], F32R, tag="oh")
                        nc.vector.tensor_scalar(
                            out=oh[:], in0=iota_f[:], scalar1=d_f[:, :1],
                            scalar2=None, op0=OP.is_equal)
                        ohT_ps = trp.tile([P, P], FP32, tag="tr")
                        nc.tensor.transpose(out=ohT_ps[:],
                                            in_=oh[:].bitcast(FP32),
                                            identity=ident32[:])
                        ohTs = sb.tile([P, P], F32R, tag="ohTs")
                        nc.vector.tensor_copy(ohTs[:], ohT_ps[:])

                        # ---- XR gathered per edge via one-hot matmul ----
                        xre_ps = mmp.tile([P, HC], FP32, tag="mm")
                        nc.tensor.matmul(out=xre_ps[:], lhsT=ohTs[:],
                                         rhs=xr_sb[:], start=True, stop=True)

                        # ---- m = G + XRe + w * We ; attention logits ----
                        t0 = sb.tile([P, HC], FP32, tag="t0")
                        nc.vector.tensor_add(t0[:], g_sb[:], xre_ps[:])
                        m_sb = sb.tile([P, HC], FP32, tag="m_sb")
                        nc.vector.scalar_tensor_tensor(
                            out=m_sb[:], in0=wes[:], scalar=wcol, in1=t0[:],
                            op0=OP.mult, op1=OP.add)
                        lrm = sb.tile([P, HC], FP32, tag="lrm")
                        nc.scalar.activation(lrm[:], m_sb[:], AF.Prelu,
                                             alpha=0.2)
                        am = sb.tile([P, HC], FP32, tag="am")
                        nc.vector.tensor_mul(am[:], lrm[:], atts[:])
                        alpha = sb.tile([P, 2], FP32, tag="alpha")
                        nc.vector.reduce_sum(
                            out=alpha[:],
                            in_=am[:].rearrange("p (h c) -> p h c", c=CH),
                            axis=mybir.AxisListType.X)
                        pexp = sb.tile([P, 2], FP32, tag="pexp")
                        nc.scalar.activation(pexp[:], alpha[:], AF.Exp)

                        # ---- weighted values + den (+ wsum for layer 1) ----
                        v = sb.tile([P, 260], F32R, tag="v")
                        nc.vector.tensor_scalar_mul(v[:, 0:CH], g_sb[:, 0:CH],
                                                    pexp[:, 0:1])
                        nc.vector.tensor_scalar_mul(v[:, CH:HC], g_sb[:, CH:HC],
                                                    pexp[:, 1:2])
                        nc.vector.tensor_copy(v[:, HC:HC + 2], pexp[:])
                        ncols = 260
                        if layer == 1:
                            nc.vector.tensor_copy(
                                v[:, HC + 2:HC + 4],
                                wcol.to_broadcast([P, 2]))
                        else:
                            nc.vector.tensor_copy(v[:, HC + 2:HC + 4], pexp[:])
                        nc.tensor.matmul(out=acc[:, 0:ncols], lhsT=oh[:],
                                         rhs=v[:, 0:ncols],
                                         start=(b == 0), stop=(b == nb - 1))

                    # ---------------- tile finalize ----------------
                    den = sb.tile([P, 2], FP32, tag="den")
                    nc.vector.tensor_scalar_add(den[:], acc[:, HC:HC + 2], 1e-16)
                    rden = sb.tile([P, 2], FP32, tag="rden")
                    nc.vector.reciprocal(rden[:], den[:])
                    hp = sb.tile([P, HC], FP32, tag="hp")
                    nc.vector.tensor_scalar_mul(hp[:, 0:CH], acc[:, 0:CH],
                                                rden[:, 0:1])
                    nc.vector.tensor_scalar_mul(hp[:, CH:HC], acc[:, CH:HC],
                                                rden[:, 1:2])
                    if layer == 1:
                        ivd = sb.tile([P, 1], FP32, tag="ivd")
                        nc.sync.dma_start(out=ivd[:],
                                          in_=invdT[t * P:(t + 1) * P, :])
                        lat = sb.tile([P, 1], FP32, tag="lat")
                        nc.vector.tensor_mul(lat[:], acc[:, HC + 2:HC + 3],
                                             ivd[:])
                        slt = sb.tile([P, 1], I32, tag="slt")
                        nc.sync.dma_start(out=slt[:],
                                          in_=slotsT[t * P:(t + 1) * P, :])
                        nc.gpsimd.indirect_dma_start(
                            out=w2i[:, :],
                            out_offset=bass.IndirectOffsetOnAxis(
                                ap=slt[:, :1], axis=0),
                            in_=lat[:], in_offset=None)
                    # stats
                    sq = sb.tile([P, HC], FP32, tag="sq")
                    nc.scalar.activation(sq[:], hp[:], AF.Square)
                    s1_ps = mmp.tile([P, HC], FP32, tag="mm")
                    nc.tensor.matmul(out=s1_ps[0:1, :], lhsT=ones_col[:],
                                     rhs=hp[:], start=True, stop=True)
                    s2_ps = mmp.tile([P, HC], FP32, tag="mm")
                    nc.tensor.matmul(out=s2_ps[0:1, :], lhsT=ones_col[:],
                                     rhs=sq[:], start=True, stop=True)
                    nc.vector.tensor_add(stsb[0:1, 0:HC], stsb[0:1, 0:HC],
                                         s1_ps[0:1, :])
                    nc.vector.tensor_add(stsb[0:1, HC:2 * HC],
                                         stsb[0:1, HC:2 * HC], s2_ps[0:1, :])
                    nc.sync.dma_start(out=hpre_dram[t * P:(t + 1) * P, :],
                                      in_=hp[:])

            # ============ BN finalize: AllReduce stats + scale/shift ============
            def bn_scale_shift(stsb, sti, sto, grow_s, brow_s):
                nc.sync.dma_start(out=sti[:, :], in_=stsb[:])
                nc.gpsimd.collective_compute(
                    "AllReduce", OP.add, ins=[sti[:, :]], outs=[sto[:, :]],
                    replica_groups=RG)
                stg = sb.tile([1, 2 * HC], FP32, tag="stg")
                nc.sync.dma_start(out=stg[:], in_=sto[:, :])
                mu = sb.tile([1, HC], FP32, tag="mu")
                nc.vector.tensor_scalar_mul(mu[:], stg[0:1, 0:HC], 1.0 / N)
                msq = sb.tile([1, HC], FP32, tag="msq")
                nc.vector.tensor_scalar_mul(msq[:], stg[0:1, HC:2 * HC], 1.0 / N)
                musq = sb.tile([1, HC], FP32, tag="musq")
                nc.vector.tensor_mul(musq[:], mu[:], mu[:])
                var = sb.tile([1, HC], FP32, tag="var")
                nc.vector.tensor_sub(var[:], msq[:], musq[:])
                nc.vector.tensor_scalar_add(var[:], var[:], 1e-5)
                sd = sb.tile([1, HC], FP32, tag="sd")
                nc.scalar.activation(sd[:], var[:], AF.Sqrt)
                rsd = sb.tile([1, HC], FP32, tag="rsd")
                nc.vector.reciprocal(rsd[:], sd[:])
                scl = sb.tile([1, HC], FP32, tag="scl")
                nc.vector.tensor_mul(scl[:], grow_s[:], rsd[:])
                sclmu = sb.tile([1, HC], FP32, tag="sclmu")
                nc.vector.tensor_mul(sclmu[:], scl[:], mu[:])
                shf = sb.tile([1, HC], FP32, tag="shf")
                nc.vector.tensor_sub(shf[:], brow_s[:], sclmu[:])
                # replicate across partitions via K=1 matmul
                scl_ps = mmp.tile([P, HC], FP32, tag="mm")
                nc.tensor.matmul(out=scl_ps[:], lhsT=ones_row[:], rhs=scl[:],
                                 start=True, stop=True)
                sclb = cst.tile([P, HC], FP32, tag=f"sclb{id(stsb)}")
                nc.scalar.activation(sclb[:], scl_ps[:], AF.Copy)
                shf_ps = mmp.tile([P, HC], FP32, tag="mm")
                nc.tensor.matmul(out=shf_ps[:], lhsT=ones_row[:], rhs=shf[:],
                                 start=True, stop=True)
                shfb = cst.tile([P, HC], FP32, tag=f"shfb{id(stsb)}")
                nc.scalar.activation(shfb[:], shf_ps[:], AF.Copy)
                return sclb, shfb

            # ================= run the whole network =================
            edge_layer(1)
            scl1b, shf1b = bn_scale_shift(st1sb, st1i, st1o, g1s, be1s)

            # BN1 apply + leaky relu + dropout mask -> h1act
            for t in range(NT):
                ht = sb.tile([P, HC], FP32, tag="ht")
                nc.sync.dma_start(out=ht[:], in_=hpre1[t * P:(t + 1) * P, :])
                t1 = sb.tile([P, HC], FP32, tag="t1")
                nc.vector.tensor_mul(t1[:], ht[:], scl1b[:])
                nc.vector.tensor_add(t1[:], t1[:], shf1b[:])
                ha = sb.tile([P, HC], FP32, tag="ha")
                nc.scalar.activation(ha[:], t1[:], AF.Prelu, alpha=0.01)
                mk = sb.tile([P, HC], FP32, tag="mk")
                nc.sync.dma_start(out=mk[:], in_=mask1T[t * P:(t + 1) * P, :])
                hm = sb.tile([P, HC], FP32, tag="hm")
                nc.vector.tensor_mul(hm[:], ha[:], mk[:])
                nc.sync.dma_start(out=h1act[t * P:(t + 1) * P, :], in_=hm[:])
                if h1dbg is not None:
                    nc.sync.dma_start(out=h1dbg[t * P:(t + 1) * P, :], in_=hm[:])

            # all-gather activated layer-1 features
            nc.gpsimd.collective_compute(
                "AllGather", OP.bypass, ins=[h1act[:, :]], outs=[h1full[:, :]],
                replica_groups=RG)

            edge_layer(2)
            scl2b, shf2b = bn_scale_shift(st2sb, st2i, st2o, g2s, be2s)

            # BN2 apply + leaky relu + dropout + FC
            for t in range(NT):
                ht = sb.tile([P, HC], FP32, tag="ht2")
                nc.sync.dma_start(out=ht[:], in_=hpre2[t * P:(t + 1) * P, :])
                t1 = sb.tile([P, HC], FP32, tag="t12")
                nc.vector.tensor_mul(t1[:], ht[:], scl2b[:])
                nc.vector.tensor_add(t1[:], t1[:], shf2b[:])
                ha = sb.tile([P, HC], FP32, tag="ha2")
                nc.scalar.activation(ha[:], t1[:], AF.Prelu, alpha=0.01)
                mk = sb.tile([P, HC], FP32, tag="mk2")
                nc.sync.dma_start(out=mk[:], in_=mask2T[t * P:(t + 1) * P, :])
                hm = sb.tile([P, HC], FP32, tag="hm2")
                nc.vector.tensor_mul(hm[:], ha[:], mk[:])
                h2T = sb.tile([P, HC], F32R, tag="h2T")
                for kk in range(2):
                    tp = trp.tile([P, P], FP32, tag="tr")
                    nc.tensor.transpose(out=tp[:],
                                        in_=hm[:, kk * P:(kk + 1) * P],
                                        identity=ident32[:])
                    nc.vector.tensor_copy(h2T[:, kk * P:(kk + 1) * P], tp[:])
                fc_ps = mmp.tile([P, HC], FP32, tag="mm")
                for kk in range(2):
                    nc.tensor.matmul(out=fc_ps[:, 0:NCF],
                                     lhsT=h2T[:, kk * P:(kk + 1) * P],
                                     rhs=wfcs[:, kk * NCF:(kk + 1) * NCF],
                                     start=(kk == 0), stop=(kk == 1))
                ob = sb.tile([P, NCF], FP32, tag="ob")
                nc.vector.tensor_add(ob[:], fc_ps[:, 0:NCF], bfcs[:])
                nc.sync.dma_start(out=outT[t * P:(t + 1) * P, :], in_=ob[:])

    nc.compile()
    return nc


# ----------------------------------------------------------------------------
# entry point
# ----------------------------------------------------------------------------

def kernel(x_input, edge_weight, params, edge_index):
    in_maps, meta = _host_prep(x_input, edge_weight, params, edge_index)
    key = tuple(sorted(meta.items()))
    if key not in _PROGRAM_CACHE:
        _PROGRAM_CACHE[key] = _build_program(meta)
    nc = _PROGRAM_CACHE[key]
    res = run_bass_kernel_spmd(nc, in_maps, core_ids=list(range(NCORES)))
    if os.environ.get("KERNEL_DBG"):
        kernel.last_res = res
        kernel.last_meta = meta
    if res.exec_time_ns is not None:
        print(f"HW exec time: {res.exec_time_ns} ns")
    out = np.empty((N, NCF), np.float32)
    for r in range(NCORES):
        out[r * NOWN:(r + 1) * NOWN] = res.results[r]["out"][:NOWN]
    return out


# revision 20
# speedup vs baseline: 1.5168x; 1.5168x over previous
"""Trainium2 Bass kernel for nn_AttnGCN (2-layer GATv2 + BN + dropout + FC).

Sharding: nodes are partitioned across 8 NeuronCores (graph parallel).  Each
core owns a contiguous range of 6250 destination nodes (padded to 6272 =
49*128).  Edges are bucketed by destination tile on the host (index-only
preprocessing), each tile's edge list padded to whole 128-edge blocks.  Layer-1
runs per-core on the edge shard; BN statistics are combined with a tiny
AllReduce; the activated layer-1 features are AllGathered (bf16) so every core
can gather arbitrary source rows for layer-2; layer-2 + FC produce the owned
output shard, which the host concatenates.

All numeric work (matmuls, softmax, scatter/gather, BN, masking) happens on
device.  Host does only index bucketing, parameter layout, and output
reassembly.  Dropout masks are the fixed jax PRNG streams of the reference
(input-independent constants), computed once on host CPU.
"""

import os
import sys
import types
import numpy as np
import ml_dtypes

import concourse.bacc as bacc
import concourse.bass as bass
import concourse.mybir as mybir
import concourse.tile as tile
from concourse.bass_utils import run_bass_kernel_spmd
from concourse.masks import make_identity

P = 128
NCORES = 8
N = 50000
E = 400000
NCF = 26          # input/output feature dim
H = 2
CH = 128
HC = 256
NOWN = N // NCORES            # 6250 owned nodes per core
NT = (NOWN + P - 1) // P      # 49 node tiles per core
NPC = NT * P                  # 6272 padded nodes per core
NPAD = NCORES * NPC           # 50176
NREAL_LAST = NOWN - (NT - 1) * P   # 106 real nodes in last tile
DP_SCALE = 1.25               # 1/(1-0.2)
KA = NCF + 1                  # 27: x features + edge weight (for w*We fold)

FP32 = mybir.dt.float32
F32R = mybir.dt.float32r
BF16 = mybir.dt.bfloat16
I32 = mybir.dt.int32
AF = mybir.ActivationFunctionType
OP = mybir.AluOpType
RG = [list(range(NCORES))]
BF = ml_dtypes.bfloat16

_PROGRAM_CACHE = {}
_MASK_CACHE = {}

_MASK_SCRIPT = r"""
import os, sys
for _p in reversed(os.environ.get("NIX_PYTHONPATH", "").split(os.pathsep)):
    if _p and _p not in sys.path:
        sys.path.insert(0, _p)
import numpy as np
import jax
m1 = np.asarray(jax.random.bernoulli(jax.random.key(1), 0.8, (%d, %d)),
                dtype=np.float32)
m2 = np.asarray(jax.random.bernoulli(jax.random.key(2), 0.8, (%d, %d)),
                dtype=np.float32)
np.savez(sys.argv[1], m1=m1, m2=m2)
"""


def _dropout_masks():
    """Reference dropout masks: fixed jax PRNG streams, computed with plain
    CPU jax (subprocess) so the bit stream matches a stock jax environment."""
    if "m" not in _MASK_CACHE:
        import subprocess
        import tempfile
        env = dict(os.environ)
        env["JAX_PLATFORMS"] = "cpu"
        env.pop("XLA_FLAGS", None)
        env.pop("TRN_TERMINAL_POOL_IPS", None)
        with tempfile.TemporaryDirectory() as td:
            fn = os.path.join(td, "masks.npz")
            script = _MASK_SCRIPT % (N, HC, N, HC)
            r = subprocess.run([sys.executable, "-c", script, fn], env=env,
                               capture_output=True, text=True)
            if r.returncode != 0:
                raise RuntimeError("mask subprocess failed: " + r.stderr[-2000:])
            d = np.load(fn)
            _MASK_CACHE["m"] = (d["m1"] * DP_SCALE, d["m2"] * DP_SCALE)
    return _MASK_CACHE["m"]


# ----------------------------------------------------------------------------
# host-side index preprocessing (sharding)
# ----------------------------------------------------------------------------

def _host_prep(x_input, edge_weight, params, edge_index):
    src = np.asarray(edge_index[0], dtype=np.int64).astype(np.int32)
    dst = np.asarray(edge_index[1], dtype=np.int64).astype(np.int32)
    w = np.asarray(edge_weight, dtype=np.float32).reshape(-1)
    x = np.asarray(x_input, dtype=np.float32)

    r_arr = dst // NOWN
    nloc = dst - r_arr * NOWN
    t_arr = nloc >> 7
    dstloc = (nloc & 127).astype(np.int32)

    key = r_arr * NT + t_arr
    order = np.argsort(key, kind="stable")
    cnt = np.bincount(key, minlength=NCORES * NT).reshape(NCORES, NT)
    bounds = np.concatenate([[0], np.cumsum(cnt.reshape(-1))]).astype(np.int64)

    # ----- layer 1 blocks -----
    NBT1 = np.maximum(1, -(-cnt.max(0) // P)).astype(np.int64)
    off1 = np.concatenate([[0], np.cumsum(NBT1[:-1])]).astype(np.int64)
    LB1 = int(NBT1.sum())
    LE1 = LB1 * P
    edges1 = np.zeros((NCORES, LE1, 3), np.int32)
    edges1[:, :, 1] = 255

    for r in range(NCORES):
        for t in range(NT):
            k = r * NT + t
            c = int(cnt[r, t])
            if c:
                sl = order[bounds[k]:bounds[k] + c]
                base = int(off1[t]) * P
                edges1[r, base:base + c, 0] = src[sl]
                edges1[r, base:base + c, 1] = dstloc[sl]
                edges1[r, base:base + c, 2] = w[sl].view(np.int32)

    # ----- layer 2 blocks (real edges + self loops) -----
    nreal = np.full(NT, P, np.int64)
    nreal[NT - 1] = NREAL_LAST
    cnt2 = cnt + nreal[None, :]
    NBT2 = np.maximum(1, -(-cnt2.max(0) // P)).astype(np.int64)
    off2 = np.concatenate([[0], np.cumsum(NBT2[:-1])]).astype(np.int64)
    LB2 = int(NBT2.sum())
    LE2 = LB2 * P
    LE2P = LE2 + P
    g2src = (src // NOWN) * NPC + src % NOWN   # padded-global source ids

    edges2 = np.zeros((NCORES, LE2, 2), np.int32)
    edges2[:, :, 1] = 255
    w2x = np.zeros((NCORES, LE2P), np.float32)
    slots2 = np.zeros((NCORES, NT * P), np.int32)

    for r in range(NCORES):
        for t in range(NT):
            k = r * NT + t
            c = int(cnt[r, t])
            sl = order[bounds[k]:bounds[k] + c]
            base = int(off2[t]) * P
            edges2[r, base:base + c, 0] = g2src[sl]
            edges2[r, base:base + c, 1] = dstloc[sl]
            w2x[r, base:base + c] = w[sl]
            nr = int(nreal[t])
            pos = base + c
            edges2[r, pos:pos + nr, 0] = r * NPC + t * P + np.arange(nr)
            edges2[r, pos:pos + nr, 1] = np.arange(nr)
            slots2[r, t * P:t * P + nr] = pos + np.arange(nr)
            slots2[r, t * P + nr:(t + 1) * P] = LE2 + np.arange(nr, P)

    # ----- degrees / masks / params -----
    deg = np.bincount(dst, minlength=N).astype(np.float32)
    invdeg_full = 1.0 / np.maximum(deg, 1.0)
    invdeg = np.ones((NCORES, NPC), np.float32)
    for r in range(NCORES):
        invdeg[r, :NOWN] = invdeg_full[r * NOWN:(r + 1) * NOWN]

    m1, m2 = _dropout_masks()

    def shard_rows(a, dtype=np.float32):
        out = np.zeros((NCORES, NPC) + a.shape[1:], dtype)
        for r in range(NCORES):
            out[r, :NOWN] = a[r * NOWN:(r + 1) * NOWN]
        return out

    mask1 = shard_rows(m1)
    mask2 = shard_rows(m2)
    x_own = shard_rows(x, BF)

    p = {k: np.asarray(v, dtype=np.float32) for k, v in params.items()}
    wfc_pk = np.zeros((P, 52), BF)
    wfc_pk[:, :26] = p["Wfc"][:128].astype(BF)
    wfc_pk[:, 26:] = p["Wfc"][128:].astype(BF)
    common = {
        "x": x,
        "Wl1": p["Wl1"].astype(BF),
        "we1row": p["We1"].reshape(1, HC).astype(BF),
        "Wr1": p["Wr1"].astype(BF),
        "att1r": np.tile(p["att1"].reshape(1, HC), (P, 1)).astype(BF),
        "Wl2": p["Wl2"].astype(BF), "Wr2": p["Wr2"].astype(BF),
        "we2row": p["We2"].reshape(1, HC).astype(BF),
        "att2r": np.tile(p["att2"].reshape(1, HC), (P, 1)).astype(BF),
        "wfc": wfc_pk,
        "bfcr": np.tile(p["bfc"].reshape(1, NCF), (P, 1)),
        "g1row": p["g1"].reshape(1, HC), "be1row": p["be1"].reshape(1, HC),
        "g2row": p["g2"].reshape(1, HC), "be2row": p["be2"].reshape(1, HC),
    }
    in_maps = []
    for r in range(NCORES):
        m = dict(common)
        m["edges1"] = edges1[r]
        m["w1c"] = edges1[r][:, 2].view(np.float32).copy()[:, None]
        m["edges2"] = edges2[r]
        m["w2x"] = w2x[r][:, None]
        m["slots2"] = slots2[r][:, None]
        m["invdeg"] = invdeg[r][:, None]
        m["mask1"] = mask1[r]
        m["mask2"] = mask2[r]
        m["x_own"] = x_own[r]
        in_maps.append(m)

    meta = dict(NBT1=tuple(int(v) for v in NBT1), off1=tuple(int(v) for v in off1),
                NBT2=tuple(int(v) for v in NBT2), off2=tuple(int(v) for v in off2),
                LE1=LE1, LE2=LE2, LE2P=LE2P)
    return in_maps, meta


# ----------------------------------------------------------------------------
# device program
# ----------------------------------------------------------------------------

def _build_program(meta):
    NBT1, off1 = meta["NBT1"], meta["off1"]
    NBT2, off2 = meta["NBT2"], meta["off2"]
    LE1, LE2, LE2P = meta["LE1"], meta["LE2"], meta["LE2P"]

    nc = bacc.Bacc("TRN2", target_bir_lowering=False)

    # ---- I/O ----
    xT = nc.dram_tensor("x", (N, NCF), FP32, kind="ExternalInput")
    e1T = nc.dram_tensor("edges1", (LE1, 3), I32, kind="ExternalInput")
    e2T = nc.dram_tensor("edges2", (LE2, 2), I32, kind="ExternalInput")
    w2xT = nc.dram_tensor("w2x", (LE2P, 1), FP32, kind="ExternalInput")
    slotsT = nc.dram_tensor("slots2", (NT * P, 1), I32, kind="ExternalInput")
    invdT = nc.dram_tensor("invdeg", (NPC, 1), FP32, kind="ExternalInput")
    mask1T = nc.dram_tensor("mask1", (NPC, HC), FP32, kind="ExternalInput")
    mask2T = nc.dram_tensor("mask2", (NPC, HC), FP32, kind="ExternalInput")
    xownT = nc.dram_tensor("x_own", (NPC, NCF), BF16, kind="ExternalInput")
    Wl1T = nc.dram_tensor("Wl1", (NCF, HC), BF16, kind="ExternalInput")
    we1rowT = nc.dram_tensor("we1row", (1, HC), BF16, kind="ExternalInput")
    w1cT = nc.dram_tensor("w1c", (LE1, 1), FP32, kind="ExternalInput")
    Wr1T = nc.dram_tensor("Wr1", (NCF, HC), BF16, kind="ExternalInput")
    att1rT = nc.dram_tensor("att1r", (P, HC), BF16, kind="ExternalInput")
    Wl2T = nc.dram_tensor("Wl2", (HC, HC), BF16, kind="ExternalInput")
    Wr2T = nc.dram_tensor("Wr2", (HC, HC), BF16, kind="ExternalInput")
    we2rowT = nc.dram_tensor("we2row", (1, HC), BF16, kind="ExternalInput")
    att2rT = nc.dram_tensor("att2r", (P, HC), BF16, kind="ExternalInput")
    wfcT = nc.dram_tensor("wfc", (P, 52), BF16, kind="ExternalInput")
    bfcrT = nc.dram_tensor("bfcr", (P, NCF), FP32, kind="ExternalInput")
    g1rT = nc.dram_tensor("g1row", (1, HC), FP32, kind="ExternalInput")
    be1rT = nc.dram_tensor("be1row", (1, HC), FP32, kind="ExternalInput")
    g2rT = nc.dram_tensor("g2row", (1, HC), FP32, kind="ExternalInput")
    be2rT = nc.dram_tensor("be2row", (1, HC), FP32, kind="ExternalInput")
    outT = nc.dram_tensor("out", (NPC, NCF), FP32, kind="ExternalOutput")

    # ---- internal DRAM ----
    dbg = bool(os.environ.get("KERNEL_DBG"))
    dbgkind = {"kind": "ExternalOutput"} if dbg else {}
    hpre1 = nc.dram_tensor("hpre1", (NPC, HC), FP32, **dbgkind)
    h1act = nc.dram_tensor("h1act", (NPC, HC), BF16)
    h1dbg = (nc.dram_tensor("h1dbg", (NPC, HC), FP32, kind="ExternalOutput")
             if dbg else None)
    h1full = nc.dram_tensor("h1full", (NPAD, HC), BF16, addr_space="Shared")
    hpre2 = nc.dram_tensor("hpre2", (NPC, HC), FP32, **dbgkind)
    w2i = nc.dram_tensor("w2i", (LE2P, 1), FP32, **dbgkind)
    st1i = nc.dram_tensor("st1i", (1, 2 * HC), FP32)
    st1o = nc.dram_tensor("st1o", (1, 2 * HC), FP32, addr_space="Shared")
    st2i = nc.dram_tensor("st2i", (1, 2 * HC), FP32)
    st2o = nc.dram_tensor("st2o", (1, 2 * HC), FP32, addr_space="Shared")

    with tile.TileContext(nc) as tc:
        with tc.tile_pool(name="cst", bufs=1) as cst, \
             tc.tile_pool(name="sb", bufs=4) as sb, \
             tc.tile_pool(name="mm", bufs=3, space="PSUM") as mmp, \
             tc.tile_pool(name="tr", bufs=3, space="PSUM") as trp, \
             tc.tile_pool(name="accp", bufs=2, space="PSUM") as accp:

            # ---------------- constants ----------------
            iota_i = cst.tile([P, P], I32, tag="iota_i")
            nc.gpsimd.iota(iota_i[:], pattern=[[1, P]], base=0,
                           channel_multiplier=0)
            iota_f = cst.tile([P, P], FP32, tag="iota_f")
            nc.vector.tensor_copy(iota_f[:], iota_i[:])
            ident32 = cst.tile([P, P], FP32, tag="ident32")
            make_identity(nc, ident32[:])
            identb = cst.tile([P, P], BF16, tag="identb")
            nc.vector.tensor_copy(identb[:], ident32[:])

            def load_const(name, dram, shape, dtype=FP32):
                t = cst.tile(shape, dtype, tag=name)
                nc.sync.dma_start(out=t[:], in_=dram[:, :])
                return t

            Wl1s = load_const("Wl1s", Wl1T, [NCF, HC], BF16)
            we1rs = load_const("we1rs", we1rowT, [1, HC], BF16)
            Wr1s = load_const("Wr1s", Wr1T, [NCF, HC], BF16)
            att1s = load_const("att1s", att1rT, [P, HC], BF16)
            we2rs = load_const("we2rs", we2rowT, [1, HC], BF16)
            att2s = load_const("att2s", att2rT, [P, HC], BF16)
            wfcs = load_const("wfcs", wfcT, [P, 52], BF16)
            bfcs = load_const("bfcs", bfcrT, [P, NCF])
            g1s = load_const("g1s", g1rT, [1, HC])
            be1s = load_const("be1s", be1rT, [1, HC])
            g2s = load_const("g2s", g2rT, [1, HC])
            be2s = load_const("be2s", be2rT, [1, HC])
            Wl2s = cst.tile([P, 2 * HC], BF16, tag="Wl2s")
            Wr2s = cst.tile([P, 2 * HC], BF16, tag="Wr2s")
            for kk in range(2):
                nc.sync.dma_start(out=Wl2s[:, kk * HC:(kk + 1) * HC],
                                  in_=Wl2T[kk * P:(kk + 1) * P, :])
                nc.sync.dma_start(out=Wr2s[:, kk * HC:(kk + 1) * HC],
                                  in_=Wr2T[kk * P:(kk + 1) * P, :])
            ones_col = cst.tile([P, 1], FP32, tag="ones_col")
            nc.vector.memset(ones_col[:], 1.0)
            ones_row = cst.tile([1, P], FP32, tag="ones_row")
            nc.vector.memset(ones_row[:], 1.0)

            st1sb = cst.tile([1, 2 * HC], FP32, tag="st1sb")
            nc.vector.memset(st1sb[:], 0.0)
            st2sb = cst.tile([1, 2 * HC], FP32, tag="st2sb")
            nc.vector.memset(st2sb[:], 0.0)

            # copy host edge weights for layer 2 (self-loop slots get filled
            # by the device during layer-1 finalize)
            nrows = LE2P // P
            for c0 in range(0, nrows, P):
                cn = min(P, nrows - c0)
                w2cp = sb.tile([P, P], FP32, tag="w2cp")
                nc.sync.dma_start(
                    out=w2cp[:cn, :],
                    in_=w2xT[:, 0].rearrange("(a b) -> a b", b=P)[c0:c0 + cn, :])
                nc.sync.dma_start(
                    out=w2i[:, 0].rearrange("(a b) -> a b", b=P)[c0:c0 + cn, :],
                    in_=w2cp[:cn, :])

            # ================= generic GATv2 edge layer =================
            def edge_layer(layer):
                if layer == 1:
                    NBT, off, eT = NBT1, off1, e1T
                    atts = att1s
                    hpre_dram = hpre1
                    stsb = st1sb
                    ew = 3   # ints per edge record
                else:
                    NBT, off, eT = NBT2, off2, e2T
                    atts = att2s
                    hpre_dram = hpre2
                    stsb = st2sb
                    ew = 2

                for t in range(NT):
                    nb = NBT[t]
                    # ---- XR tile for the 128 owned nodes ----
                    if layer == 1:
                        xo = sb.tile([P, NCF], BF16, tag="xo")
                        nc.sync.dma_start(out=xo[:],
                                          in_=xownT[t * P:(t + 1) * P, :])
                        xoT_ps = trp.tile([P, P], BF16, tag="tr")
                        nc.tensor.transpose(out=xoT_ps[:NCF, :], in_=xo[:],
                                            identity=identb[:])
                        xoTs = sb.tile([NCF, P], BF16, tag="xoTs")
                        nc.vector.tensor_copy(xoTs[:], xoT_ps[:NCF, :])
                        xr_ps = mmp.tile([P, HC], FP32, tag="mm")
                        nc.tensor.matmul(out=xr_ps[:], lhsT=xoTs[:],
                                         rhs=Wr1s[:], start=True, stop=True)
                    else:
                        xo = sb.tile([P, HC], BF16, tag="xo2")
                        nc.sync.dma_start(out=xo[:],
                                          in_=h1act[t * P:(t + 1) * P, :])
                        xoTs = sb.tile([P, HC], BF16, tag="xoTs2")
                        for kk in range(2):
                            tp = trp.tile([P, P], BF16, tag="tr")
                            nc.tensor.transpose(
                                out=tp[:], in_=xo[:, kk * P:(kk + 1) * P],
                                identity=identb[:])
                            nc.vector.tensor_copy(
                                xoTs[:, kk * P:(kk + 1) * P], tp[:])
                        xr_ps = mmp.tile([P, HC], FP32, tag="mm")
                        for kk in range(2):
                            nc.tensor.matmul(
                                out=xr_ps[:],
                                lhsT=xoTs[:, kk * P:(kk + 1) * P],
                                rhs=Wr2s[:, kk * HC:(kk + 1) * HC],
                                start=(kk == 0), stop=(kk == 1))
                    xr_sb = sb.tile([P, HC], BF16, tag="xr_sb")
                    nc.scalar.activation(xr_sb[:], xr_ps[:], AF.Copy)

                    # ---- whole tile's edge records in one DMA ----
                    o0 = off[t] * P
                    ebt = sb.tile([P, nb * ew], I32, tag="ebt")
                    nc.sync.dma_start(
                        out=ebt[:].rearrange("p (b c) -> p b c", c=ew),
                        in_=eT[o0:o0 + nb * P, :].rearrange(
                            "(b p) c -> p b c", p=P))
                    wsrc = w1cT if layer == 1 else w2i
                    wrf = sb.tile([1, nb * P], FP32, tag="wrf")
                    nc.sync.dma_start(
                        out=wrf[:],
                        in_=wsrc[o0:o0 + nb * P, :].rearrange("a one -> one a"))
                    wrowt = sb.tile([1, nb * P], BF16, tag="wrowt")
                    nc.vector.tensor_copy(wrowt[:], wrf[:])

                    acc = accp.tile([P, 260], FP32, tag="acc")
                    for b in range(nb):
                        sidx = ebt[:, b * ew:b * ew + 1]
                        didx = ebt[:, b * ew + 1:b * ew + 2]

                        # ---- gather + source transform -> G [128, 256] ----
                        if layer == 1:
                            xg = sb.tile([P, NCF], BF16, tag="xg")
                            nc.gpsimd.indirect_dma_start(
                                out=xg[:], out_offset=None, in_=xT[:, :],
                                in_offset=bass.IndirectOffsetOnAxis(
                                    ap=sidx, axis=0))
                            xgT_ps = trp.tile([P, P], BF16, tag="tr")
                            nc.tensor.transpose(out=xgT_ps[:NCF, :],
                                                in_=xg[:],
                                                identity=identb[:])
                            xgTs = sb.tile([NCF, P], BF16, tag="xgTs")
                            nc.vector.tensor_copy(xgTs[:], xgT_ps[:NCF, :])
                            g_ps = mmp.tile([P, HC], FP32, tag="mm")
                            nc.tensor.matmul(out=g_ps[:], lhsT=xgTs[:],
                                             rhs=Wl1s[:], start=True, stop=True)
                        else:
                            grow = sb.tile([P, HC], BF16, tag="grow")
                            nc.gpsimd.indirect_dma_start(
                                out=grow[:], out_offset=None, in_=h1full[:, :],
                                in_offset=bass.IndirectOffsetOnAxis(
                                    ap=sidx, axis=0))
                            gTs = sb.tile([P, HC], BF16, tag="gTs")
                            for kk in range(2):
                                tp = trp.tile([P, P], BF16, tag="tr")
                                nc.tensor.transpose(
                                    out=tp[:], in_=grow[:, kk * P:(kk + 1) * P],
                                    identity=identb[:])
                                nc.vector.tensor_copy(
                                    gTs[:, kk * P:(kk + 1) * P], tp[:])
                            g_ps = mmp.tile([P, HC], FP32, tag="mm")
                            for kk in range(2):
                                nc.tensor.matmul(
                                    out=g_ps[:],
                                    lhsT=gTs[:, kk * P:(kk + 1) * P],
                                    rhs=Wl2s[:, kk * HC:(kk + 1) * HC],
                                    start=(kk == 0), stop=(kk == 1))
                        g_sb = sb.tile([P, HC], BF16, tag="g_sb")
                        nc.scalar.activation(g_sb[:], g_ps[:], AF.Copy)

                        # ---- one-hot by local destination ----
                        d_f = sb.tile([P, 1], FP32, tag="d_f")
                        nc.vector.tensor_copy(d_f[:], didx)
                        oh = sb.tile([P, P], BF16, tag="oh")
                        nc.vector.tensor_scalar(
                            out=oh[:], in0=iota_f[:], scalar1=d_f[:, :1],
                            scalar2=None, op0=OP.is_equal)
                        ohT_ps = trp.tile([P, P], BF16, tag="tr")
                        nc.tensor.transpose(out=ohT_ps[:], in_=oh[:],
                                            identity=identb[:])
                        ohTs = sb.tile([P, P], BF16, tag="ohTs")
                        nc.vector.tensor_copy(ohTs[:], ohT_ps[:])

                        # ---- XRe (+ w*We for layer 2) via one-hot matmul ----
                        xre_ps = mmp.tile([P, HC], FP32, tag="mm")
                        wers = we1rs if layer == 1 else we2rs
                        nc.tensor.matmul(out=xre_ps[:], lhsT=ohTs[:],
                                         rhs=xr_sb[:], start=True, stop=False)
                        nc.tensor.matmul(out=xre_ps[:],
                                         lhsT=wrowt[:, b * P:(b + 1) * P],
                                         rhs=wers[:], start=False, stop=True)

                        # ---- m = G + XRe(+wWe); attention logits ----
                        m_sb = sb.tile([P, HC], BF16, tag="m_sb")
                        nc.vector.tensor_add(m_sb[:], g_sb[:], xre_ps[:])
                        lrm = sb.tile([P, HC], BF16, tag="lrm")
                        nc.scalar.activation(lrm[:], m_sb[:], AF.Prelu,
                                             alpha=0.2)
                        junk = sb.tile([P, P], BF16, tag="junk")
                        alpha = sb.tile([P, 2], FP32, tag="alpha")
                        for hh in range(2):
                            nc.vector.scalar_tensor_tensor(
                                out=junk[:],
                                in0=lrm[:, hh * CH:(hh + 1) * CH], scalar=1.0,
                                in1=atts[:, hh * CH:(hh + 1) * CH],
                                op0=OP.mult, op1=OP.mult,
                                accum_out=alpha[:, hh:hh + 1])
                        pexp = sb.tile([P, 2], FP32, tag="pexp")
                        nc.scalar.activation(pexp[:], alpha[:], AF.Exp)

                        # ---- weighted values + den (+ wsum for layer 1) ----
                        v = sb.tile([P, 260], BF16, tag="v")
                        nc.vector.tensor_scalar_mul(v[:, 0:CH], g_sb[:, 0:CH],
                                                    pexp[:, 0:1])
                        nc.vector.tensor_scalar_mul(v[:, CH:HC], g_sb[:, CH:HC],
                                                    pexp[:, 1:2])
                        nc.vector.tensor_copy(v[:, HC:HC + 2], pexp[:])
                        if layer == 1:
                            nc.vector.tensor_copy(
                                v[:, HC + 2:HC + 4],
                                ebt[:, b * ew + 2:b * ew + 3]
                                .bitcast(FP32).to_broadcast([P, 2]))
                        else:
                            nc.vector.tensor_copy(v[:, HC + 2:HC + 4], pexp[:])
                        nc.tensor.matmul(out=acc[:, 0:260], lhsT=oh[:],
                                         rhs=v[:, 0:260],
                                         start=(b == 0), stop=(b == nb - 1))

                    # ---------------- tile finalize ----------------
                    den = sb.tile([P, 2], FP32, tag="den")
                    nc.vector.tensor_scalar_add(den[:], acc[:, HC:HC + 2], 1e-16)
                    rden = sb.tile([P, 2], FP32, tag="rden")
                    nc.vector.reciprocal(rden[:], den[:])
                    hp = sb.tile([P, HC], FP32, tag="hp")
                    nc.vector.tensor_scalar_mul(hp[:, 0:CH], acc[:, 0:CH],
                                                rden[:, 0:1])
                    nc.vector.tensor_scalar_mul(hp[:, CH:HC], acc[:, CH:HC],
                                                rden[:, 1:2])
                    if layer == 1:
                        ivd = sb.tile([P, 1], FP32, tag="ivd")
                        nc.sync.dma_start(out=ivd[:],
                                          in_=invdT[t * P:(t + 1) * P, :])
                        lat = sb.tile([P, 1], FP32, tag="lat")
                        nc.vector.tensor_mul(lat[:], acc[:, HC + 2:HC + 3],
                                             ivd[:])
                        slt = sb.tile([P, 1], I32, tag="slt")
                        nc.sync.dma_start(out=slt[:],
                                          in_=slotsT[t * P:(t + 1) * P, :])
                        nc.gpsimd.indirect_dma_start(
                            out=w2i[:, :],
                            out_offset=bass.IndirectOffsetOnAxis(
                                ap=slt[:, :1], axis=0),
                            in_=lat[:], in_offset=None)
                    # stats
                    sq = sb.tile([P, HC], FP32, tag="sq")
                    nc.scalar.activation(sq[:], hp[:], AF.Square)
                    s1_ps = mmp.tile([P, HC], FP32, tag="mm")
                    nc.tensor.matmul(out=s1_ps[0:1, :], lhsT=ones_col[:],
                                     rhs=hp[:], start=True, stop=True)
                    s2_ps = mmp.tile([P, HC], FP32, tag="mm")
                    nc.tensor.matmul(out=s2_ps[0:1, :], lhsT=ones_col[:],
                                     rhs=sq[:], start=True, stop=True)
                    nc.vector.tensor_add(stsb[0:1, 0:HC], stsb[0:1, 0:HC],
                                         s1_ps[0:1, :])
                    nc.vector.tensor_add(stsb[0:1, HC:2 * HC],
                                         stsb[0:1, HC:2 * HC], s2_ps[0:1, :])
                    nc.sync.dma_start(out=hpre_dram[t * P:(t + 1) * P, :],
                                      in_=hp[:])

            # ============ BN finalize: AllReduce stats + scale/shift ============
            def bn_scale_shift(stsb, sti, sto, grow_s, brow_s):
                nc.sync.dma_start(out=sti[:, :], in_=stsb[:])
                nc.gpsimd.collective_compute(
                    "AllReduce", OP.add, ins=[sti[:, :]], outs=[sto[:, :]],
                    replica_groups=RG)
                stg = sb.tile([1, 2 * HC], FP32, tag="stg")
                nc.sync.dma_start(out=stg[:], in_=sto[:, :])
                mu = sb.tile([1, HC], FP32, tag="mu")
                nc.vector.tensor_scalar_mul(mu[:], stg[0:1, 0:HC], 1.0 / N)
                msq = sb.tile([1, HC], FP32, tag="msq")
                nc.vector.tensor_scalar_mul(msq[:], stg[0:1, HC:2 * HC], 1.0 / N)
                musq = sb.tile([1, HC], FP32, tag="musq")
                nc.vector.tensor_mul(musq[:], mu[:], mu[:])
                var = sb.tile([1, HC], FP32, tag="var")
                nc.vector.tensor_sub(var[:], msq[:], musq[:])
                nc.vector.tensor_scalar_add(var[:], var[:], 1e-5)
                sd = sb.tile([1, HC], FP32, tag="sd")
                nc.scalar.activation(sd[:], var[:], AF.Sqrt)
                rsd = sb.tile([1, HC], FP32, tag="rsd")
                nc.vector.reciprocal(rsd[:], sd[:])
                scl = sb.tile([1, HC], FP32, tag="scl")
                nc.vector.tensor_mul(scl[:], grow_s[:], rsd[:])
                sclmu = sb.tile([1, HC], FP32, tag="sclmu")
                nc.vector.tensor_mul(sclmu[:], scl[:], mu[:])
                shf = sb.tile([1, HC], FP32, tag="shf")
                nc.vector.tensor_sub(shf[:], brow_s[:], sclmu[:])
                scl_ps = mmp.tile([P, HC], FP32, tag="mm")
                nc.tensor.matmul(out=scl_ps[:], lhsT=ones_row[:], rhs=scl[:],
                                 start=True, stop=True)
                sclb = cst.tile([P, HC], FP32, tag=f"sclb{id(stsb)}")
                nc.scalar.activation(sclb[:], scl_ps[:], AF.Copy)
                shf_ps = mmp.tile([P, HC], FP32, tag="mm")
                nc.tensor.matmul(out=shf_ps[:], lhsT=ones_row[:], rhs=shf[:],
                                 start=True, stop=True)
                shfb = cst.tile([P, HC], FP32, tag=f"shfb{id(stsb)}")
                nc.scalar.activation(shfb[:], shf_ps[:], AF.Copy)
                return sclb, shfb

            # ================= run the whole network =================
            edge_layer(1)
            scl1b, shf1b = bn_scale_shift(st1sb, st1i, st1o, g1s, be1s)

            # BN1 apply + leaky relu + dropout mask -> h1act (bf16)
            for t in range(NT):
                ht = sb.tile([P, HC], FP32, tag="ht")
                nc.sync.dma_start(out=ht[:], in_=hpre1[t * P:(t + 1) * P, :])
                t1 = sb.tile([P, HC], FP32, tag="t1")
                nc.vector.tensor_mul(t1[:], ht[:], scl1b[:])
                nc.vector.tensor_add(t1[:], t1[:], shf1b[:])
                ha = sb.tile([P, HC], FP32, tag="ha")
                nc.scalar.activation(ha[:], t1[:], AF.Prelu, alpha=0.01)
                mk = sb.tile([P, HC], FP32, tag="mk")
                nc.sync.dma_start(out=mk[:], in_=mask1T[t * P:(t + 1) * P, :])
                hm = sb.tile([P, HC], BF16, tag="hm")
                nc.vector.tensor_mul(hm[:], ha[:], mk[:])
                nc.sync.dma_start(out=h1act[t * P:(t + 1) * P, :], in_=hm[:])
                if h1dbg is not None:
                    hmf = sb.tile([P, HC], FP32, tag="hmf")
                    nc.vector.tensor_copy(hmf[:], hm[:])
                    nc.sync.dma_start(out=h1dbg[t * P:(t + 1) * P, :], in_=hmf[:])

            # all-gather activated layer-1 features (bf16)
            nc.gpsimd.collective_compute(
                "AllGather", OP.bypass, ins=[h1act[:, :]], outs=[h1full[:, :]],
                replica_groups=RG)

            edge_layer(2)
            scl2b, shf2b = bn_scale_shift(st2sb, st2i, st2o, g2s, be2s)

            # BN2 apply + leaky relu + dropout + FC
            for t in range(NT):
                ht = sb.tile([P, HC], FP32, tag="ht2")
                nc.sync.dma_start(out=ht[:], in_=hpre2[t * P:(t + 1) * P, :])
                t1 = sb.tile([P, HC], FP32, tag="t12")
                nc.vector.tensor_mul(t1[:], ht[:], scl2b[:])
                nc.vector.tensor_add(t1[:], t1[:], shf2b[:])
                ha = sb.tile([P, HC], FP32, tag="ha2")
                nc.scalar.activation(ha[:], t1[:], AF.Prelu, alpha=0.01)
                mk = sb.tile([P, HC], FP32, tag="mk2")
                nc.sync.dma_start(out=mk[:], in_=mask2T[t * P:(t + 1) * P, :])
                hm = sb.tile([P, HC], BF16, tag="hm2")
                nc.vector.tensor_mul(hm[:], ha[:], mk[:])
                h2T = sb.tile([P, HC], BF16, tag="h2T")
                for kk in range(2):
                    tp = trp.tile([P, P], BF16, tag="tr")
                    nc.tensor.transpose(out=tp[:],
                                        in_=hm[:, kk * P:(kk + 1) * P],
                                        identity=identb[:])
                    nc.vector.tensor_copy(h2T[:, kk * P:(kk + 1) * P], tp[:])
                fc_ps = mmp.tile([P, HC], FP32, tag="mm")
                for kk in range(2):
                    nc.tensor.matmul(out=fc_ps[:, 0:NCF],
                                     lhsT=h2T[:, kk * P:(kk + 1) * P],
                                     rhs=wfcs[:, kk * NCF:(kk + 1) * NCF],
                                     start=(kk == 0), stop=(kk == 1))
                ob = sb.tile([P, NCF], FP32, tag="ob")
                nc.vector.tensor_add(ob[:], fc_ps[:, 0:NCF], bfcs[:])
                nc.sync.dma_start(out=outT[t * P:(t + 1) * P, :], in_=ob[:])

    nc.compile()
    return nc


# ----------------------------------------------------------------------------
# entry point
# ----------------------------------------------------------------------------

def kernel(x_input, edge_weight, params, edge_index):
    in_maps, meta = _host_prep(x_input, edge_weight, params, edge_index)
    key = tuple(sorted(meta.items()))
    if key not in _PROGRAM_CACHE:
        _PROGRAM_CACHE[key] = _build_program(meta)
    nc = _PROGRAM_CACHE[key]
    res = run_bass_kernel_spmd(nc, in_maps, core_ids=list(range(NCORES)))
    if os.environ.get("KERNEL_DBG"):
        kernel.last_res = res
        kernel.last_meta = meta
    if res.exec_time_ns is not None:
        print(f"HW exec time: {res.exec_time_ns} ns")
    out = np.empty((N, NCF), np.float32)
    for r in range(NCORES):
        out[r * NOWN:(r + 1) * NOWN] = res.results[r]["out"][:NOWN]
    return out
